# revision 4
# baseline (speedup 1.0000x reference)
"""3-layer GCN (PyG GCNConv-style) on 8 Trainium2 NeuronCores.

Distribution: 1-D node partition (2048 nodes per core). Per core:
  - GEMM1: h1T[36,2048] = W1s^T @ x[rows_c]^T in fp8 e4m3 with DoubleRow
    perf mode (K=256 per matmul, 0.5 cyc/col). W1 is pre-scaled by 64 on
    the host so fp8 quantization stays in the normal range.
  - Per layer: messages g = (dis/64) * hT are computed in fp16, PE-transposed
    to node-major, cast to fp8, AllGathered (fp8), and aggregated with a
    dense per-core adjacency block A[16384, 2048] fp8 (exact small edge
    multiplicities; symmetric norm folded into pre/post dis scaling) via
    fp8 DoubleRow matmuls accumulating aggT[36, 2048] in PSUM.
  - A is split: the first NRES column-pairs stay resident in SBUF for all
    3 layers; the rest re-stream each layer (streamed first so their DMA
    hides under the collective).
  - Small GEMMs (W2, W3 pre-scaled by 512, fp16), f32 softmax tail.

Scaling discipline (host folds all constants):
  W1s=64*W1, W2s=512*W2, W3s=512*W3, disr_pre=dis/64, disr_post=dis,
  b2s=8*b2. Then every fp8 message tensor has rms ~0.25 and the final
  logits come out exactly right: logits = disr_pre * aggT3 + b3.
"""
import numpy as np
import concourse.bacc as bacc
import concourse.mybir as mybir
import concourse.tile as tile
from concourse.bass_utils import run_bass_kernel_spmd

N = 16384
E = 524288
H = 36
C = 16
NCORES = 8
ND = N // NCORES          # 2048 nodes per core
NP2 = 64                  # chunk-pairs (K = 256 each)
RB = ND // 128            # 16 row-blocks per core
HP = 48                   # H padded to 16B multiple (dual-fp8 LdWeights step alignment)
NRES = 30                 # resident A chunk-pairs (tuned to SBUF capacity)
NSTREAM = 6               # rotating stream buffers
FP8 = mybir.dt.np(mybir.dt.float8e4)

_PROGRAM = None
_LAST_RES = None


def _build_program(nres=NRES):
    nc = bacc.Bacc(None)
    f32, f16, fp8 = mybir.dt.float32, mybir.dt.float16, mybir.dt.float8e4
    DR = mybir.MatmulPerfMode.DoubleRow

    xT_d = nc.dram_tensor("xT", [NP2, 128, 2, ND], fp8, kind="ExternalInput")
    W1c_d = nc.dram_tensor("W1c", [128, NP2, 2, HP], fp8, kind="ExternalInput")
    A_d = nc.dram_tensor("A", [NP2, 128, 2, ND], fp8, kind="ExternalInput")
    dpre_d = nc.dram_tensor("dpre", [H, ND], f32, kind="ExternalInput")
    dpost_d = nc.dram_tensor("dpost", [H, ND], f32, kind="ExternalInput")
    W2_d = nc.dram_tensor("W2s", [H, H], f16, kind="ExternalInput")
    W3_d = nc.dram_tensor("W3s", [H, C], f16, kind="ExternalInput")
    b1_d = nc.dram_tensor("b1", [H, 1], f32, kind="ExternalInput")
    b2_d = nc.dram_tensor("b2s", [H, 1], f32, kind="ExternalInput")
    b3_d = nc.dram_tensor("b3", [C, 1], f32, kind="ExternalInput")
    I16_d = nc.dram_tensor("ident16", [128, 128], f16, kind="ExternalInput")
    I32_d = nc.dram_tensor("ident32", [128, 128], f32, kind="ExternalInput")
    out_d = nc.dram_tensor("out", [ND, C], f32, kind="ExternalOutput")

    cc_in = [nc.dram_tensor(f"cc{l}_in", [ND, HP], fp8)
             for l in range(3)]
    cc_out = [nc.dram_tensor(f"cc{l}_out", [N, HP], fp8,
                             addr_space="Shared")
              for l in range(3)]
    groups = [list(range(NCORES))]

    with tile.TileContext(nc) as tc:
        with (
            tc.tile_pool(name="const", bufs=1) as constp,
            tc.tile_pool(name="ares", bufs=1) as aresp,
            tc.tile_pool(name="stream", bufs=NSTREAM) as streamp,
            tc.tile_pool(name="gt", bufs=1) as gtp,
            tc.tile_pool(name="work", bufs=1) as work,
            tc.tile_pool(name="psb", bufs=1, space="PSUM") as psb,
            tc.tile_pool(name="pst", bufs=2, space="PSUM") as pst,
        ):
            W1c = constp.tile([128, NP2, 2, HP], fp8)
            dpre = constp.tile([H, ND], f32)
            dpost = constp.tile([H, ND], f32)
            W2t = constp.tile([H, H], f16)
            W3t = constp.tile([H, C], f16)
            b1t = constp.tile([H, 1], f32)
            b2t = constp.tile([H, 1], f32)
            b3t = constp.tile([C, 1], f32)
            ident16 = constp.tile([128, 128], f16)
            ident32 = constp.tile([128, 128], f32)
            nc.sync.dma_start(W1c[:], W1c_d[:])
            nc.sync.dma_start(dpre[:], dpre_d[:])
            nc.sync.dma_start(dpost[:], dpost_d[:])
            nc.sync.dma_start(W2t[:], W2_d[:])
            nc.sync.dma_start(W3t[:], W3_d[:])
            nc.sync.dma_start(b1t[:], b1_d[:])
            nc.sync.dma_start(b2t[:], b2_d[:])
            nc.sync.dma_start(b3t[:], b3_d[:])
            nc.sync.dma_start(ident16[:], I16_d[:])
            nc.sync.dma_start(ident32[:], I32_d[:])

            A_res = aresp.tile([128, nres, 2, ND], fp8)

            # ---- GEMM1: hT[36, 2048] += W1s[pair]^T @ xT[pair] (fp8 DR) ----
            with nc.named_scope("gemm1"):
                hT = psb.tile([H, ND], f32, tag="big")
                for c2 in range(NP2):
                    xt = streamp.tile([128, 2, ND], fp8, tag="st")
                    nc.sync.dma_start(xt[:], xT_d[c2, :, :, :])
                    for q in range(4):
                        nc.tensor.matmul(
                            hT[:, q * 512:(q + 1) * 512],
                            W1c[:, c2, :, 0:H],
                            xt[:, :, q * 512:(q + 1) * 512],
                            start=(c2 == 0),
                            stop=(c2 == NP2 - 1),
                            perf_mode=DR,
                        )

            # resident A loads (program order after gemm1 issue; DMA deps free)
            for r in range(nres):
                nc.sync.dma_start(A_res[:, r, :, :], A_d[r, :, :, :])

            for layer in range(3):
                F = H if layer < 2 else C
                # ---- stage: g = dpre*hT -> transpose -> fp8 -> AllGather ----
                with nc.named_scope(f"stage{layer}"):
                    g16 = work.tile([H, ND], f16, tag="g16")
                    nc.vector.tensor_tensor(
                        g16[0:F, :], hT[0:F, :], dpre[0:F, :],
                        mybir.AluOpType.mult,
                    )
                    gown = work.tile([128, RB, HP], fp8, tag="gown")
                    for rb in range(RB):
                        tp = pst.tile([128, H], f16, tag="tpA")
                        nc.tensor.transpose(
                            tp[:, 0:F],
                            g16[0:F, rb * 128:(rb + 1) * 128],
                            ident16[0:F, 0:F],
                        )
                        nc.scalar.activation(
                            gown[:, rb, 0:F], tp[:, 0:F],
                            mybir.ActivationFunctionType.Copy,
                        )
                    nc.sync.dma_start(
                        cc_in[layer][:].rearrange("(b p) f -> p b f", p=128),
                        gown[:],
                    )
                    nc.gpsimd.collective_compute(
                        "AllGather",
                        mybir.AluOpType.bypass,
                        replica_groups=groups,
                        ins=[cc_in[layer][:]],
                        outs=[cc_out[layer][:]],
                    )
                    g_t = gtp.tile([128, NP2, 2, HP], fp8, tag="g")
                    nc.sync.dma_start(
                        g_t[:],
                        cc_out[layer][:].rearrange(
                            "(p c j) f -> p c j f", p=128, j=2
                        ),
                    )

                # ---- aggregation: aggT[F, 2048] += g[pair]^T @ A[pair] ----
                with nc.named_scope(f"agg{layer}"):
                    aggT = psb.tile([H, ND], f32, tag="big")
                    order = list(range(nres, NP2)) + list(range(nres))
                    for idx, c2 in enumerate(order):
                        if c2 >= nres:
                            a_t = streamp.tile([128, 2, ND], fp8, tag="st")
                            nc.sync.dma_start(a_t[:], A_d[c2, :, :, :])
                            rhs = a_t
                        else:
                            rhs = A_res[:, c2, :, :]
                        for q in range(4):
                            nc.tensor.matmul(
                                aggT[0:F, q * 512:(q + 1) * 512],
                                g_t[:, c2, :, 0:F],
                                rhs[:, :, q * 512:(q + 1) * 512],
                                start=(idx == 0),
                                stop=(idx == NP2 - 1),
                                perf_mode=DR,
                            )

                with nc.named_scope(f"post{layer}"):
                    if layer < 2:
                        # in_{l+1} = relu(dpost*aggT + b); next hT = W^T @ in
                        tmp16 = work.tile([H, ND], f16, tag="tmp16")
                        nc.vector.tensor_tensor(
                            tmp16[:], aggT[0:H, :], dpost[:],
                            mybir.AluOpType.mult,
                        )
                        inx = work.tile([H, ND], f16, tag="inx")
                        nc.scalar.activation(
                            inx[:], tmp16[:], mybir.ActivationFunctionType.Relu,
                            bias=b1t[:] if layer == 0 else b2t[:],
                        )
                        Wt = W2t if layer == 0 else W3t
                        Fn = H if layer == 0 else C
                        hT = psb.tile([H, ND], f32, tag="big")
                        for q in range(4):
                            nc.tensor.matmul(
                                hT[0:Fn, q * 512:(q + 1) * 512],
                                Wt[:, 0:Fn],
                                inx[:, q * 512:(q + 1) * 512],
                                start=True,
                                stop=True,
                            )
                    else:
                        # logits = dpre*aggT + b3; softmax over classes
                        tmpL = work.tile([C, ND], f32, tag="tmpL")
                        nc.vector.tensor_tensor(
                            tmpL[:], aggT[0:C, :], dpre[0:C, :],
                            mybir.AluOpType.mult,
                        )
                        logT = work.tile([C, ND], f32, tag="logT")
                        nc.vector.tensor_scalar(
                            logT[:], tmpL[:], b3t[:], None,
                            mybir.AluOpType.add,
                        )
                        onat = work.tile([128, RB, C], f32, tag="onat")
                        for rb in range(RB):
                            tp2 = pst.tile([128, C], f32, tag="tpB")
                            nc.tensor.transpose(
                                tp2[:, 0:C],
                                logT[:, rb * 128:(rb + 1) * 128],
                                ident32[0:C, 0:C],
                            )
                            nc.vector.tensor_copy(onat[:, rb, :], tp2[:, 0:C])
                        negmax = work.tile([128, RB], f32, tag="negmax")
                        nc.vector.tensor_reduce(
                            negmax[:], onat[:], axis=mybir.AxisListType.X,
                            op=mybir.AluOpType.max, negate=True,
                        )
                        expv = work.tile([128, RB, C], f32, tag="expv")
                        ssum = work.tile([128, RB], f32, tag="ssum")
                        for rb in range(RB):
                            nc.scalar.activation(
                                expv[:, rb, :], onat[:, rb, :],
                                mybir.ActivationFunctionType.Exp,
                                bias=negmax[:, rb:rb + 1],
                                accum_out=ssum[:, rb:rb + 1],
                            )
                        rsum = work.tile([128, RB], f32, tag="rsum")
                        nc.vector.reciprocal(rsum[:], ssum[:])
                        prob = work.tile([128, RB, C], f32, tag="prob")
                        for rb in range(RB):
                            nc.vector.tensor_scalar(
                                prob[:, rb, :], expv[:, rb, :],
                                rsum[:, rb:rb + 1], None,
                                mybir.AluOpType.mult,
                            )
                        nc.sync.dma_start(
                            out_d[:].rearrange("(b p) f -> p b f", p=128),
                            prob[:],
                        )

    nc.finalize()
    return nc


def _get_program():
    global _PROGRAM
    if _PROGRAM is None:
        _PROGRAM = _build_program()
    return _PROGRAM


def kernel(x, edge_index, W1, b1, W2, b2, W3, b3, _profile=False):
    x = np.asarray(x, dtype=np.float32)
    edge_index = np.asarray(edge_index)
    W1 = np.asarray(W1, dtype=np.float32)
    W2 = np.asarray(W2, dtype=np.float32)
    W3 = np.asarray(W3, dtype=np.float32)
    b1 = np.asarray(b1, dtype=np.float32)
    b2 = np.asarray(b2, dtype=np.float32)
    b3 = np.asarray(b3, dtype=np.float32)

    # ---- graph preprocessing (host) ----
    loop = np.arange(N, dtype=np.int64)
    src = np.concatenate([edge_index[0].astype(np.int64), loop])
    dst = np.concatenate([edge_index[1].astype(np.int64), loop])
    deg = np.bincount(dst, minlength=N).astype(np.float32)
    dis = (1.0 / np.sqrt(np.maximum(deg, np.float32(1.0)))).astype(np.float32)

    order = np.argsort(dst)
    src_s, dst_s = src[order], dst[order]
    core_of = dst_s // ND
    bounds = np.searchsorted(core_of, np.arange(NCORES + 1))

    W1c = np.zeros((128, NP2, 2, HP), dtype=FP8)
    W1c[:, :, :, 0:H] = (W1 * np.float32(64.0)).astype(FP8).reshape(
        128, NP2, 2, H)
    W2s = (W2 * np.float32(512.0)).astype(np.float16)
    W3s = (W3 * np.float32(512.0)).astype(np.float16)
    ident16 = np.eye(128, dtype=np.float16)
    ident32 = np.eye(128, dtype=np.float32)

    xT32 = np.ascontiguousarray(x.T)  # [k, node] fp32

    in_maps = []
    for c in range(NCORES):
        lo, hi = bounds[c], bounds[c + 1]
        Af = np.zeros((N, ND), dtype=np.float32)
        np.add.at(Af, (src_s[lo:hi], dst_s[lo:hi] - c * ND), np.float32(1.0))
        A8 = np.ascontiguousarray(
            Af.astype(FP8).reshape(128, NP2, 2, ND).transpose(1, 0, 2, 3)
        )
        xc8 = xT32[:, c * ND:(c + 1) * ND].astype(FP8)  # [16384, 2048]
        xT8 = np.ascontiguousarray(
            xc8.reshape(128, NP2, 2, ND).transpose(1, 0, 2, 3)
        )
        disc = dis[c * ND:(c + 1) * ND]
        dpre = np.ascontiguousarray(
            np.broadcast_to((disc / np.float32(64.0))[None, :], (H, ND))
        ).astype(np.float32)
        dpost = np.ascontiguousarray(
            np.broadcast_to(disc[None, :], (H, ND))
        ).astype(np.float32)
        in_maps.append({
            "xT": xT8,
            "W1c": W1c,
            "A": A8,
            "dpre": dpre,
            "dpost": dpost,
            "W2s": W2s,
            "W3s": W3s,
            "b1": b1.reshape(H, 1),
            "b2s": (b2 * np.float32(8.0)).reshape(H, 1),
            "b3": b3.reshape(C, 1),
            "ident16": ident16,
            "ident32": ident32,
        })

    nc = _get_program()
    global _LAST_RES
    res = run_bass_kernel_spmd(nc, in_maps, list(range(NCORES)),
                               trace=bool(_profile))
    _LAST_RES = res
    out = np.concatenate([res.results[c]["out"] for c in range(NCORES)], axis=0)
    if _profile:
        return out, res.exec_time_ns
    return out
